# revision 11
# baseline (speedup 1.0000x reference)
"""Trainium2 Bass kernel: visibility prediction (softplus -> 3x3 Hann conv ->
exact type-2 NuDFT), sharded over nvis across 8 NeuronCores.

vis[k] = cell^2 * sum_{y,x} img[y,x] exp(-2i*pi*u_k*c_x) exp(-2i*pi*v_k*c_y)

Compute structure (folded symmetric NuDFT) is unchanged from the baseline:
  - Pixel coords fold the image along x and y into sum/difference parts, so
    phases are needed only for 128 positive multiples per axis and the
    x-contraction collapses to 128-row matmul blocks.
  - All matmuls in float32r (single-pass PE mode).
  - Phase args produced on PE with the magic-number rounding trick
    ([q, +M, -M, -q] rows accumulate to round(q)-q); ACT only evaluates Sin.
  - Final y-reduction as tensor_tensor_reduce with center-pixel terms via
    the reduce init scalar.

The end-to-end time is dominated by the axon tunnel round-trip (~45 ms
fixed) plus ~2 ms per 128 KB moved, so the wrapper is built around ONE jit
dispatch per call with minimal payload (modeled on-device exec: ~112 us):
  - uu/vv ship as int16 fixed point (round(u*phs*2^17), 200 KB total,
    sharded over nvis); int16 -> f32r value-converting DMAs restore them
    on device, and the 6-row PE operand layout (data rows + ones rows +
    pad) is overlaid on a NEFF-baked all-ones template.
  - All PE-side constants (jrow, magic rows, templates) are NEFF Const
    tensors (inline_tensor) — zero per-call transfer.
  - base_cube ships row-sharded bf16 (16 KB/core, 128 KB total instead of
    2 MB f32 replicated) and is reassembled on device with an in-kernel
    HBM AllGather (the neuronx_cc bass_exec hook forbids any other HLO op
    in the module, so the collective must live inside the Bass kernel).
  - Output zero-buffer operands are persistent device arrays (never
    transferred; the kernel writes every output element).
  - Visibilities return as f16 scaled by 2^50 (f16 normal range), 200 KB
    total; the host unscales during assembly.
"""

import sys

if "/opt/trn_rl_repo" not in sys.path:
    sys.path.insert(0, "/opt/trn_rl_repo")

import numpy as np
from contextlib import ExitStack

import concourse.bass as bass  # noqa: F401
import concourse.tile as tile
from concourse import bacc, mybir
from concourse import masks
from concourse.ap import AP

NCORES = 8
NPIX = 256
NVIS = 50000
NV_CORE = NVIS // NCORES            # 6250
NCHUNK = (NV_CORE + 127) // 128     # 49
NV_PAD = NCHUNK * 128               # 6272
KBATCH = 256                        # u-phase batch (2 chunks)

CELL = np.float32(0.005) * np.float32(np.pi / 180.0 / 3600.0)
PHS = np.float32(np.float32(1000.0) * CELL)  # kilolambda -> cycles/pixel
# conv computed as (0.5*l + c + 0.5*r) per axis = 4x the Hann weights;
# fold the 1/4 together with the cell^2 pixel solid angle into one scalar.
# OSHIFT: visibilities ship as f16 to halve the d2h payload; 2^50 recenters
# their ~1e-11..1e-17 magnitudes into f16 normal range (host divides back).
OSHIFT = 50
SCALE = float(np.float32(np.float64(CELL) ** 2 / 4.0 * 2.0 ** OSHIFT))
F32 = mybir.dt.float32
F32R = mybir.dt.float32r
F16 = mybir.dt.float16
BF16 = mybir.dt.bfloat16
I16 = mybir.dt.int16
PI = float(np.pi)
# u/v phases ship as int16 fixed point: up_int = round(up * 2^17) with
# up in [-0.25, 0.25] cycles/pixel. The PE then works in int units
# (products of <=8-bit x <=16-bit mantissas stay exact), the rounding
# bias moves to 1.5*2^40 (f32 ulp there = 2^17 = one cycle), the 0.25
# cos shift becomes exactly 2^15, and the Sin activation scale absorbs
# the 2^-17.
QBITS = 17
QSCALE = float(2.0 ** QBITS)
MAGIC = float(np.float32(1.5 * 2 ** (23 + QBITS)))  # round-to-nearest bias

_CACHE = {}


def _ap2(base, col0, blk):
    """[2x128] view of adjacent 128-col blocks starting at col0."""
    return AP(tensor=base.tensor, offset=base.offset + col0,
              ap=[list(base.ap[0]), [blk, 2], [1, 128]])


def _build():
    AF = mybir.ActivationFunctionType
    OP = mybir.AluOpType
    nc = bacc.Bacc("TRN2", target_bir_lowering=False, debug=False,
                   num_devices=NCORES)
    NROW = NPIX // NCORES               # 32 base_cube rows per core
    uv_ap = nc.dram_tensor("uv", [1, 2 * NV_CORE], I16,
                           kind="ExternalInput").ap()
    bc_ap = nc.dram_tensor("bc_shard", [NROW, NPIX], BF16,
                           kind="ExternalInput").ap()
    out_ap = nc.dram_tensor("out_ri", [2, 128, NCHUNK], F16,
                            kind="ExternalOutput").ap()
    u_ap = uv_ap[:, 0:NV_CORE]
    v_ap = uv_ap[:, NV_CORE:2 * NV_CORE]

    # --- constants baked into the NEFF (no per-call transfer) ---
    # jrow stays exact small integers: the f32r single-pass PE multiply
    # loses precision on full-mantissa x full-mantissa products, so the
    # kilolambda->cycles/pixel scale is applied to u/v on the host instead
    # of being folded in here.
    jrow = np.arange(1, 129, dtype=np.float32)
    Mrow = np.full(128, MAGIC, np.float32)
    qrow = np.full(128, np.float32(2.0 ** (QBITS - 2)), np.float32)
    zrow = np.zeros(128, np.float32)
    jr6_np = np.stack([jrow, zrow, Mrow, -Mrow, -jrow, zrow])
    jw6_np = np.stack([jrow, qrow, Mrow, -Mrow, -jrow, -qrow])
    mr6_np = np.zeros((6, 256), np.float32)
    mr6_np[0, 0:128] = jrow
    mr6_np[0, 128:256] = jrow
    mr6_np[1, 128:256] = np.float32(2.0 ** (QBITS - 2))
    mr6_np[2, :] = MAGIC
    mr6_np[3, :] = -MAGIC
    mr6_np[4, 0:128] = -jrow
    mr6_np[4, 128:256] = -jrow
    mr6_np[5, 128:256] = -np.float32(2.0 ** (QBITS - 2))
    jr6_ap = nc.inline_tensor(jr6_np, name="jr6c").ap()
    jw6_ap = nc.inline_tensor(jw6_np, name="jw6c").ap()
    mr6_ap = nc.inline_tensor(mr6_np, name="mr6c").ap()
    on_ap = nc.inline_tensor(np.ones((1, 128), np.float32), name="onesc").ap()
    t6_ap = nc.inline_tensor(np.ones((6, NV_PAD), np.float32),
                             name="t6ones").ap()

    with tile.TileContext(nc) as tc, ExitStack() as ctx:
        persist = ctx.enter_context(tc.tile_pool(name="persist", bufs=1))
        u6 = persist.tile([6, NV_PAD], F32R, tag="u6")
        v6 = persist.tile([6, NV_PAD], F32R, tag="v6")
        jr6 = persist.tile([6, 128], F32R, tag="jr6")
        jw6 = persist.tile([6, 128], F32R, tag="jw6")
        mr6 = persist.tile([6, 256], F32R, tag="mr6")
        onesr = persist.tile([1, 128], F32R, tag="onesr")
        mv_im = persist.tile([128, 258], F32R, tag="mv_im")
        mv_re = persist.tile([128, 258], F32R, tag="mv_re")
        mv_c = persist.tile([1, 258], F32R, tag="mv_c")
        stage = persist.tile([128, 2 * NCHUNK], F32, tag="stage")
        ident = persist.tile([128, 128], F32, tag="ident")
        raws = [persist.tile([128, NPIX], F32, tag=f"raw{i}",
                             name=f"raw{i}") for i in range(2)]
        # base_cube arrives row-sharded (32 rows/core); reassemble the full
        # image with an HBM-HBM AllGather over the 8 cores (DRAM bounce
        # buffers: collectives can't touch I/O tensors directly).
        dram = ctx.enter_context(tc.tile_pool(name="dram", bufs=1,
                                              space="DRAM"))
        bc_bin = dram.tile([NROW, NPIX], BF16, tag="bc_bin")
        bc_bout = dram.tile([NPIX, NPIX], BF16, tag="bc_bout")
        nc.sync.dma_start(bc_bin[:], bc_ap[:])
        nc.gpsimd.collective_compute(
            "AllGather", mybir.AluOpType.bypass,
            replica_groups=[list(range(NCORES))],
            ins=[bc_bin.opt()], outs=[bc_bout.opt()])
        # bf16 -> f32 casting DMAs (gpsimd-initiated)
        nc.gpsimd.dma_start(raws[0][:], bc_bout[0:128, :])
        nc.gpsimd.dma_start(raws[1][:], bc_bout[128:256, :])
        # u6/v6: rows {0,4} = raw u/v, rows {1,2,3,5} = ones. DMA a baked
        # all-ones template over the whole tile (memset can't target f32r),
        # then DMA the data rows over it; the pad columns of rows 0/4 stay
        # 1.0, producing finite garbage in the discarded k >= NV_CORE
        # output slots.
        for t6, s_ap in ((u6, u_ap), (v6, v_ap)):
            nc.gpsimd.dma_start(t6[:], t6_ap[:])
            # int16 -> f32r value-converting DMAs (gpsimd-initiated)
            nc.gpsimd.dma_start(t6[0:1, 0:NV_CORE], s_ap[:])
            nc.gpsimd.dma_start(t6[4:5, 0:NV_CORE], s_ap[:])
        # f32 Const -> f32r SBUF counts as a casting DMA: gpsimd-initiated
        nc.gpsimd.dma_start(jr6[:], jr6_ap[:])
        nc.gpsimd.dma_start(jw6[:], jw6_ap[:])
        nc.gpsimd.dma_start(mr6[:], mr6_ap[:])
        nc.gpsimd.dma_start(onesr[:], on_ap[:])

        # ------- main loop (software-pipelined emission order) -------
        ups = ctx.enter_context(tc.tile_pool(name="ups", bufs=1, space="PSUM"))
        vps = ctx.enter_context(tc.tile_pool(name="vps", bufs=1, space="PSUM"))
        usb = ctx.enter_context(tc.tile_pool(name="usb", bufs=2))
        vsb = ctx.enter_context(tc.tile_pool(name="vsb", bufs=6))
        scr = ctx.enter_context(tc.tile_pool(name="scr", bufs=4))

        offs = []
        off = 0
        while off < NV_PAD:
            KB = min(KBATCH, NV_PAD - off)
            offs.append((off, KB))
            off += KB
        chunk_of = [(bi, c) for bi, (o, KB) in enumerate(offs)
                    for c in range(KB // 128)]
        NCH = len(chunk_of)
        phs, qvs, qabs, Ws, pair = {}, {}, {}, {}, [None]
        v_emitted = set()

        def emit_u_mm(bi, piece):
            off, KB = offs[bi]
            if piece == 0:
                # r-half in ONE matmul: rows [q, +M, -M, -q] accumulate
                # sequentially in-array -> round(q) - q exactly
                qab = ups.tile([128, 2 * KB], F32, tag="qab", name="qab")
                qabs[bi] = qab
                nc.tensor.matmul(qab[:, 0:KB], jr6[0:6, :],
                                 u6[0:6, off:off + KB],
                                 start=True, stop=True)
            else:
                # w-half: rows [q, +.25, +M, -M, -q, -.25] -> -w
                qab = qabs[bi]
                nc.tensor.matmul(qab[:, KB:2 * KB], jw6[0:6, :],
                                 u6[0:6, off:off + KB],
                                 start=True, stop=True)

        def emit_u_sin(bi, piece):
            # ph = sin(-2pi*[r | -w]) = [s_u | c_u], one instr per batch
            off, KB = offs[bi]
            qab = qabs[bi]
            if piece == 1:
                return
            ph = usb.tile([128, 2 * KB], F32R, tag="ph", name="ph")
            phs[bi] = ph
            nc.scalar.activation(ph[:], qab[:], AF.Sin,
                                 bias=0.0, scale=-2.0 * PI / QSCALE)

        def emit_v(gc):
            if gc in v_emitted:
                return
            v_emitted.add(gc)
            bi, c = chunk_of[gc]
            off, KB = offs[bi]
            k0 = off + c * 128
            if gc % 2 == 0:
                pair[0] = vps.tile([128, 512], F32, tag="qv2", name="qv2")
            qv = pair[0][:, (gc % 2) * 256:(gc % 2) * 256 + 256]
            nc.tensor.matmul(qv, v6[0:6, k0:k0 + 128], mr6[0:6, :],
                             start=True, stop=True)
            qvs[gc] = qv

        def emit_w(gc):
            # W: [0 | s_v | 0] at cols 0..129, [1 | c_v | 1] at 512..641.
            # The guard columns pair with the tcat center columns in the
            # 129-wide combine blocks (block stride 512 = one psum bank).
            W = vsb.tile([128, 642], F32, tag="W", name="W")
            qv = qvs.pop(gc)
            nc.gpsimd.memset(W[:, 0:130:129], 0.0)
            nc.gpsimd.memset(W[:, 512:642:129], 1.0)
            wv = W[:]
            nc.scalar.activation(
                AP(tensor=wv.tensor, offset=wv.offset + 1,
                   ap=[list(wv.ap[0]), [512, 2], [1, 128]]),
                _ap2(qv, 0, 128), AF.Sin, bias=0.0, scale=-2.0 * PI / QSCALE)
            Ws[gc] = W


        # start batch-0/1 phase matmuls on PE while DVE/ACT run image prep
        emit_u_mm(0, 0)
        emit_u_mm(0, 1)
        emit_v(0)
        emit_v(1)

        # ---------------- one-time image prep ----------------
        with tc.tile_pool(name="ssb", bufs=1) as ssb, \
             tc.tile_pool(name="sps", bufs=1, space="PSUM") as sps:
            masks.make_identity(nc, ident[:])
            # softplus = Ln(1 + Exp(x)) into x-padded tiles, then conv-x.
            # Row-half i=0 runs its element ops on DVE, i=1 on Pool.
            cx = []
            engs = (nc.vector, nc.gpsimd)
            for i in range(2):
                eng = engs[i]
                impad = ssb.tile([128, NPIX + 2], F32, tag=f"impad{i}")
                eng.memset(impad[:, 0:NPIX + 2:NPIX + 1], 0.0)
                raw = raws[i]
                expt = ssb.tile([128, NPIX], F32, tag=f"expt{i}")
                nc.scalar.activation(expt[:], raw[:], AF.Exp)
                nc.scalar.activation(impad[:, 1:NPIX + 1], expt[:],
                                     AF.Ln, bias=1.0, scale=1.0)
                t1 = ssb.tile([128, NPIX], F32, tag=f"t1_{i}")
                eng.tensor_tensor(t1[:], impad[:, 0:NPIX],
                                  impad[:, 2:NPIX + 2], op=OP.add)
                c = ssb.tile([128, NPIX], F32, tag=f"cx{i}", name=f"cx{i}")
                nc.vector.scalar_tensor_tensor(
                    c[:], t1[:], 0.5, impad[:, 1:NPIX + 1],
                    op0=OP.mult, op1=OP.add)
                cx.append(c)
            warm = ssb.tile([1, 2], F32, tag="warm")
            nc.scalar.activation(warm[:, 0:1], cx[0][0:1, 0:1], AF.Sin,
                                 bias=0.0, scale=0.001)
            # x-fold (x = free dim): j=1..127 pairs, j=128 <- x=0, center x=128
            gSp, gAp, gCp = [], [], []
            for i in range(2):
                eng = engs[i]
                gs = ssb.tile([128, 128], F32, tag=f"gSp{i}")
                eng.tensor_tensor(gs[:, 0:127], cx[i][:, 129:NPIX],
                                  cx[i][:, 127:0:-1], op=OP.add)
                eng.tensor_scalar_mul(gs[:, 127:128], cx[i][:, 0:1], 1.0)
                ga = ssb.tile([128, 128], F32, tag=f"gAp{i}")
                eng.tensor_tensor(ga[:, 0:127], cx[i][:, 129:NPIX],
                                  cx[i][:, 127:0:-1], op=OP.subtract)
                eng.tensor_scalar_mul(ga[:, 127:128], cx[i][:, 0:1], -1.0)
                gSp.append(gs)
                gAp.append(ga)
                gCp.append(cx[i][:, 128:129])
            # transpose to (j, y) layout; y-pad for conv-y
            gSt = ssb.tile([128, NPIX + 2], F32, tag="gSt")
            gAt = ssb.tile([128, NPIX + 2], F32, tag="gAt")
            gCt = ssb.tile([1, NPIX + 2], F32, tag="gCt")
            for di, (dst, src) in enumerate(((gSt, gSp), (gAt, gAp))):
                engs[di].memset(dst[:, 0:NPIX + 2:NPIX + 1], 0.0)
                for i in range(2):
                    ps = sps.tile([128, 128], F32, tag=f"pst{i}", name="ps")
                    nc.tensor.transpose(ps[:], src[i][:], ident[:])
                    nc.vector.tensor_scalar_mul(
                        dst[:, 1 + i * 128:1 + (i + 1) * 128], ps[:], 1.0)
            nc.vector.memset(gCt[:, 0:NPIX + 2:NPIX + 1], 0.0)
            for i in range(2):
                ps = sps.tile([1, 128], F32, tag=f"pstc{i}")
                nc.tensor.transpose(ps[:], gCp[i][:], ident[:])
                nc.vector.tensor_scalar_mul(
                    gCt[:, 1 + i * 128:1 + (i + 1) * 128], ps[:], 1.0)
            # conv-y along free dim
            conv = []
            for pad, part, sgn, eng in ((gSt, 128, 1.0, nc.vector),
                                        (gAt, 128, -1.0, nc.gpsimd),
                                        (gCt, 1, 1.0, nc.vector)):
                t2 = ssb.tile([part, NPIX], F32, tag=f"t2_{pad.name}")
                eng.tensor_tensor(t2[:], pad[:, 0:NPIX],
                                  pad[:, 2:NPIX + 2], op=OP.add)
                cc = ssb.tile([part, NPIX], F32, tag=f"cv_{pad.name}")
                nc.vector.scalar_tensor_tensor(
                    cc[:], t2[:], 0.5 * sgn, pad[:, 1:NPIX + 1],
                    op0=OP.mult, op1=OP.add if sgn > 0 else OP.subtract)
                conv.append(cc)
            gSc, gAc, gCc = conv
            # y-fold into the moving tensors:
            # mv_re = [SS | SS_c*SCALE | SA_n], mv_im = [AA | AS_c*SCALE | AS]
            # mv_c  = [pC | gC_c*SCALE | mC_n]
            for mv, g, s0, s1, cs, eng in (
                (mv_re, gSc, 1.0, -1.0, 1.0, nc.vector),
                (mv_im, gAc, -1.0, 1.0, 1.0, nc.gpsimd),
                (mv_c, gCc, 1.0, -1.0, 1.0, nc.vector),
            ):
                a = g[:, 129:NPIX]          # y = 129..255  (m=1..127)
                b = g[:, 127:0:-1]          # y = 127..1    (m=1..127)
                if s0 > 0:
                    eng.tensor_tensor(mv[:, 0:127], a, b, op=OP.add)
                else:
                    eng.tensor_tensor(mv[:, 0:127], a, b, op=OP.subtract)
                eng.tensor_scalar_mul(mv[:, 127:128], g[:, 0:1], 1.0
                                      if s0 > 0 else -1.0)
                eng.tensor_scalar_mul(mv[:, 128:129], g[:, 128:129], cs)
                if s1 > 0:
                    eng.tensor_tensor(mv[:, 129:256], a, b, op=OP.add)
                else:
                    eng.tensor_tensor(mv[:, 129:256], b, a, op=OP.subtract)
                eng.tensor_scalar_mul(mv[:, 256:257], g[:, 0:1], 1.0)
                eng.tensor_scalar_mul(mv[:, 257:258], g[:, 0:1], 0.0)

        tps = ctx.enter_context(tc.tile_pool(name="tps", bufs=3, space="PSUM"))
        emit_u_sin(0, 0)
        emit_u_sin(0, 1)
        for gc in range(NCH):
            bi, c = chunk_of[gc]
            off, KB = offs[bi]
            emit_w(gc)
            if gc + 2 < NCH:
                emit_v(gc + 2)
            if bi + 1 < len(offs):
                if c == 0:
                    emit_u_mm(bi + 1, 0)
                    emit_u_mm(bi + 1, 1)
                else:
                    emit_u_sin(bi + 1, 0)

            # T matmuls: bank0 = [M_im | T_im_c*S | P_im],
            #            bank1 = [P_re | T_re_c*S | M_re_n]
            tcat = tps.tile([128, 1024], F32, tag="tcat", name="tcat")
            ph = phs[bi]
            sl_s = ph[:, c * 128:(c + 1) * 128]
            sl_c = ph[:, KB + c * 128:KB + (c + 1) * 128]
            nc.tensor.matmul(tcat[:, 0:258], sl_s, mv_im[:],
                             start=True, stop=True)
            nc.tensor.matmul(tcat[:, 512:770], sl_c, mv_re[:],
                             start=True, stop=False)
            nc.tensor.matmul(tcat[:, 512:770], onesr[:], mv_c[:],
                             start=False, stop=True)

            tbase = tcat[:]
            wbase = Ws.pop(gc)[:]
            # re = SCALE*sum([M_im|T_im_c]*[s_v|0] + [P_re|T_re_c]*[c_v|1])
            d1 = scr.tile([128, 258], F32, tag="d1", name="d1")
            in0_re = AP(tensor=tbase.tensor, offset=tbase.offset,
                        ap=[list(tbase.ap[0]), [512, 2], [1, 129]])
            in1_re = AP(tensor=wbase.tensor, offset=wbase.offset + 1,
                        ap=[list(wbase.ap[0]), [512, 2], [1, 129]])
            nc.vector.scalar_tensor_tensor(
                AP(tensor=d1[:].tensor, offset=d1[:].offset,
                   ap=[list(d1[:].ap[0]), [129, 2], [1, 129]]),
                in0_re, SCALE, in1_re,
                op0=OP.mult, op1=OP.mult, accum_out=stage[:, gc:gc + 1])
            # im = SCALE*sum([T_re_c|M_re_n]*[0|s_v] + [T_im_c|P_im]*[1|c_v])
            d2 = scr.tile([128, 258], F32, tag="d2", name="d2")
            in0_im = AP(tensor=tbase.tensor, offset=tbase.offset + 640,
                        ap=[list(tbase.ap[0]), [-512, 2], [1, 129]])
            in1_im = AP(tensor=wbase.tensor, offset=wbase.offset,
                        ap=[list(wbase.ap[0]), [512, 2], [1, 129]])
            nc.vector.scalar_tensor_tensor(
                AP(tensor=d2[:].tensor, offset=d2[:].offset,
                   ap=[list(d2[:].ap[0]), [129, 2], [1, 129]]),
                in0_im, SCALE, in1_im,
                op0=OP.mult, op1=OP.mult,
                accum_out=stage[:, NCHUNK + gc:NCHUNK + gc + 1])
            if gc % 16 == 15 or gc == NCH - 1:
                lo = (gc // 16) * 16
                # f32 stage -> f16 output: casting DMAs (gpsimd-initiated)
                nc.gpsimd.dma_start(out_ap[0][:, lo:gc + 1],
                                    stage[:, lo:gc + 1])
                nc.gpsimd.dma_start(out_ap[1][:, lo:gc + 1],
                                    stage[:, NCHUNK + lo:NCHUNK + gc + 1])


    nc.compile()
    return nc


class _Runner:
    """Persistent jitted 8-core SPMD executor (jit built once, reused).

    Single jit dispatch per call: sharded numpy inputs in, device-side
    all-gathers for base_cube and the output, replicated output fetched
    once from a single shard.
    """

    def __init__(self, nc):
        import jax
        import jax.numpy as jnp
        from jax.sharding import Mesh, PartitionSpec as P, NamedSharding
        from jax import shard_map
        from concourse import bass2jax
        from concourse.bass2jax import install_neuronx_cc_hook

        install_neuronx_cc_hook()
        self.nc = nc
        partition_name = (nc.partition_id_tensor.name
                          if nc.partition_id_tensor else None)
        in_names, out_names, out_avals = [], [], []
        for alloc in nc.m.functions[0].allocations:
            if not isinstance(alloc, mybir.MemoryLocationSet):
                continue
            name = alloc.memorylocations[0].name
            if alloc.kind == "ExternalInput":
                if name != partition_name:
                    in_names.append(name)
            elif alloc.kind == "ExternalOutput":
                out_names.append(name)
                out_avals.append(jax.core.ShapedArray(
                    tuple(alloc.tensor_shape), mybir.dt.np(alloc.dtype)))
        self.in_names, self.out_names, self.out_avals = \
            in_names, out_names, out_avals
        all_names = in_names + out_names
        if partition_name is not None:
            all_names = all_names + [partition_name]

        assert in_names == ["uv", "bc_shard"], in_names
        n_params = len(in_names)

        def _body(*args):
            # neuronx_cc_hook requires the HLO module to be exactly
            # (parameters -> bass_exec custom-call): no other ops allowed,
            # and operand order must match the jit parameter order.
            operands = list(args)
            if partition_name is not None:
                operands.append(bass2jax.partition_id_tensor())
            outs = bass2jax._bass_exec_p.bind(
                *operands,
                out_avals=tuple(out_avals),
                in_names=tuple(all_names),
                out_names=tuple(out_names),
                lowering_input_output_aliases=(),
                sim_require_finite=True,
                sim_require_nnan=True,
                nc=nc,
            )
            return tuple(outs)

        devices = jax.devices()[:NCORES]
        mesh = Mesh(np.asarray(devices), ("core",))
        shard = NamedSharding(mesh, P("core"))
        n_outs = len(out_names)
        self._fn = jax.jit(
            shard_map(_body, mesh=mesh,
                      in_specs=(P("core"),) * (n_params + n_outs),
                      out_specs=(P("core"),) * n_outs,
                      check_vma=False),
            in_shardings=(shard,) * (n_params + n_outs),
            out_shardings=(shard,) * n_outs,
        )
        # Output zero-buffers: required operands of the custom call, but the
        # kernel writes every output element, so their content is never read.
        # Keep them resident on device across calls (no h2d, not donated).
        self._zeros = [
            jax.device_put(
                np.zeros((NCORES * a.shape[0], *a.shape[1:]), a.dtype), shard)
            for a in out_avals
        ]
        jax.block_until_ready(self._zeros)

    def __call__(self, uv, base):
        """uv: (8, 2*NV_CORE) f32 (pre-scaled u|v per core); base:
        (256,256) bf16 (row-sharded). Returns (8, 2, 128, NCHUNK) f16."""
        outs = self._fn(uv, base, *self._zeros)
        a = self.out_avals[0]
        return np.asarray(outs[0]).reshape(NCORES, *a.shape)


def _get_runner():
    if "runner" not in _CACHE:
        _CACHE["runner"] = _Runner(_build())
    return _CACHE["runner"]


def prep_inputs(base_cube, uu, vv):
    import ml_dtypes
    base = np.asarray(base_cube, np.float32)[0].astype(ml_dtypes.bfloat16)
    uv = np.empty((NCORES, 2 * NV_CORE), np.int16)
    for dst, src in ((uv[:, 0:NV_CORE], uu), (uv[:, NV_CORE:], vv)):
        q = np.rint(np.asarray(src, np.float32).astype(np.float64)
                    * (float(PHS) * QSCALE))
        np.clip(q, -32768, 32767, out=q)
        dst[:] = q.reshape(NCORES, NV_CORE).astype(np.int16)
    return uv, base


def assemble(out8):
    out = np.empty((1, NVIS), np.complex64)
    unscale = np.float32(2.0 ** -OSHIFT)
    for c in range(NCORES):
        ri = out8[c].astype(np.float32) * unscale   # (2, 128, NCHUNK)
        vis = (ri[0] + 1j * ri[1]).astype(np.complex64)
        flat = vis.T.reshape(-1)                # k = chunk*128 + partition
        out[0, c * NV_CORE:(c + 1) * NV_CORE] = flat[:NV_CORE]
    return out


def kernel(base_cube, uu, vv):
    runner = _get_runner()
    uv, base = prep_inputs(base_cube, uu, vv)
    return assemble(runner(uv, base))
